# revision 16
# baseline (speedup 1.0000x reference)
"""Expert-parallel grouped-MLP (MoE experts) kernel for 8 Trainium2 cores.

Problem: y = W2_e @ silu(W1_e @ x_e + b1_e) + b2_e for E=16 independent
experts (grouped 1x1 conv), B=8 batches, C=256 channels/expert, CAP=4,
L=1024 positions.

Sharding: expert-parallel - core i owns experts {2i, 2i+1}; no cross-core
communication.

v2: all operands are pre-cast to fp16 and pre-laid-out on the HOST in the
exact SBUF tile layouts, so the device does zero dtype casts and every DMA
is a fully contiguous [128, N] slab. y is written back as fp16 (host
up-casts). This halves DMA traffic, removes all DVE cast ops, roughly
halves DMA instruction count (shorter kernel-exit semaphore-clear tail),
and lets the first real matmul start ~4 us earlier.

  per (b, e) pair:  one DMA x[128, 2*1024] fp16
    layer 1: 8 m-tiles x (2k x 2n) matmuls -> PSUM[128,1024]
             ACT silu(. + b1) PSUM -> h SBUF [128, 8x1024] (fp16)
    layer 2: 2 j-tiles x (8q x 2n) accumulating matmuls -> PSUM[128,1024]
             DVE + b2 PSUM -> y SBUF fp16, one DMA out per pair
"""
import numpy as np

import concourse.tile as tile
from concourse import bacc, mybir
from concourse.bass_utils import run_bass_kernel_spmd

# Problem constants (hardcoded per contract)
B, E, C, CAP, L = 8, 16, 256, 4, 1024
F = C * CAP            # 1024 hidden per expert
NCORES = 8
EPC = E // NCORES      # 2 experts per core
P = 128                # partitions
KT = C // P            # 2 k-tiles (layer-1 contraction)
MT = F // P            # 8 m-tiles (layer-1 output partitions)
JT = C // P            # 2 j-tiles (layer-2 output partitions)
QT = F // P            # 8 q-tiles (layer-2 contraction)
NT = L // 512          # 2 n-tiles of 512 cols
NB = MT + JT           # bias columns per expert (8 m + 2 j)
N_WARMUP = 4           # dummy PE warmup matmuls

_FP32 = mybir.dt.float32
_FP16 = mybir.dt.float16


def _build():
    nc = bacc.Bacc("TRN2", target_bir_lowering=False, debug=False)

    # All tensors host-pre-arranged so each DMA is a contiguous [128, N] slab.
    xs_d = nc.dram_tensor("xs", [B, EPC, P, KT * L], _FP16, kind="ExternalInput")
    w1_d = nc.dram_tensor("w1", [EPC, KT, P, F], _FP16, kind="ExternalInput")
    w2_d = nc.dram_tensor("w2", [EPC, P, QT * C], _FP16, kind="ExternalInput")
    b_d = nc.dram_tensor("bs", [EPC, P, NB], _FP32, kind="ExternalInput")
    ys_d = nc.dram_tensor("ys", [B, EPC, P, JT * L], _FP16, kind="ExternalOutput")

    with tile.TileContext(nc) as tc:
        with (
            tc.tile_pool(name="const", bufs=1) as cpool,
            tc.tile_pool(name="x", bufs=6) as xpool,
            tc.tile_pool(name="h", bufs=2) as hpool,
            tc.tile_pool(name="y", bufs=3) as ypool,
            tc.tile_pool(name="ps", bufs=4, space="PSUM") as pspool,
        ):
            # ---- PE warmup: zero fp16 matmuls with no DMA deps ----
            wdum = cpool.tile([P, P], _FP16, tag="wdum")
            rdum = cpool.tile([P, 512], _FP16, tag="rdum")
            nc.vector.memset(wdum[:], 0.0)
            nc.vector.memset(rdum[:], 0.0)
            actdum = cpool.tile([P, 1], _FP32, tag="actdum")
            nc.scalar.activation(actdum[:], rdum[:, :1],
                                 mybir.ActivationFunctionType.Silu, bias=0.0)
            # single shared psum tile: WAW on the same bank from the same
            # engine needs no semaphores, so warmups run back-to-back and
            # keep the PE duty cycle high (earlier DVFS ramp to full clock)
            wps = pspool.tile([P, 512], _FP32, tag="ps", name="wps")
            for i in range(N_WARMUP):
                nc.tensor.matmul(wps[:], wdum[:], rdum[:],
                                 start=True, stop=True)

            # ---- weight/bias tiles ----
            # w1sb[e][k]: [128, F]   col m*128+j = output channel of m-tile m
            # w2sb[e]:    [128, QT*C] col q*C+c
            w1sb = [[cpool.tile([P, F], _FP16, tag=f"w1_{e}_{k}",
                                name=f"w1sb_{e}_{k}")
                     for k in range(KT)] for e in range(EPC)]
            w2sb = [cpool.tile([P, QT * C], _FP16, tag=f"w2_{e}",
                               name=f"w2sb_{e}")
                    for e in range(EPC)]
            bsb = cpool.tile([P, EPC * NB], _FP32, tag="b")

            def load_w1(e, k, split=1):
                for s in range(split):
                    w = F // split
                    nc.sync.dma_start(
                        w1sb[e][k][:, s * w:(s + 1) * w],
                        w1_d.ap()[e, k, :, s * w:(s + 1) * w],
                    )

            def load_w2(e):
                nc.sync.dma_start(w2sb[e][:], w2_d.ap()[e])

            def load_b(e):
                nc.sync.dma_start(
                    bsb[:, e * NB:(e + 1) * NB], b_d.ap()[e]
                )

            def load_x(b, e):
                # x loads ride the (otherwise idle) gpsimd DGE ring so the
                # startup weight loads on the sync ring run in parallel
                xt = xpool.tile([P, KT * L], _FP16, tag="x",
                                name=f"x_{b}_{e}")
                nc.gpsimd.dma_start(xt[:], xs_d.ap()[b, e])
                return xt

            # startup-critical order: first matmul needs w1[0][0] cols 0:128
            # and x[0,0] cols 0:512 -> issue exactly those two DMAs first;
            # expert-1 weights are deferred until pair 1 (needed at pair 8)
            x0 = xpool.tile([P, KT * L], _FP16, tag="x", name="x_0_0")
            nc.sync.dma_start(w1sb[0][0][:, :512], w1_d.ap()[0, 0, :, :512])
            nc.gpsimd.dma_start(x0[:, :512], xs_d.ap()[0, 0, :, :512])
            nc.gpsimd.dma_start(x0[:, 512:L], xs_d.ap()[0, 0, :, 512:L])
            nc.gpsimd.dma_start(x0[:, L:], xs_d.ap()[0, 0, :, L:])
            nc.sync.dma_start(w1sb[0][0][:, 512:], w1_d.ap()[0, 0, :, 512:])
            load_b(0)
            load_w1(0, 1)
            load_w2(0)

            def b1col(e, m):
                return bsb[:, e * NB + m: e * NB + m + 1]

            def b2col(e, j):
                return bsb[:, e * NB + MT + j: e * NB + MT + j + 1]

            # ---- per-(expert, batch) pipeline ----
            for e in range(EPC):
                for b in range(B):
                    xsb = x0 if (e == 0 and b == 0) else load_x(b, e)
                    if e == 0 and b == 1:
                        load_w1(1, 0)
                        load_w1(1, 1)
                        load_b(1)
                        load_w2(1)

                    # layer 1: h = silu(W1 @ x + b1), h[p, m*L + l]
                    hsb = hpool.tile([P, MT * L], _FP16, tag="h")
                    for m in range(MT):
                        psh = pspool.tile([P, L], _FP32, tag="ps")
                        for k in range(KT):
                            for n in range(NT):
                                nc.tensor.matmul(
                                    psh[:, n * 512:(n + 1) * 512],
                                    w1sb[e][k][:, m * P:(m + 1) * P],
                                    xsb[:, k * L + n * 512: k * L + (n + 1) * 512],
                                    start=(k == 0),
                                    stop=(k == KT - 1),
                                )
                        nc.scalar.activation(
                            hsb[:, m * L:(m + 1) * L],
                            psh[:],
                            mybir.ActivationFunctionType.Silu,
                            bias=b1col(e, m),
                        )

                    # layer 2: y = W2 @ h + b2
                    last_pair = (e == EPC - 1 and b == B - 1)
                    if last_pair:
                        # n-outer with a SEPARATE 1-bank psum tile per chunk:
                        # DVE/DMA of earlier chunks overlap the later matmul
                        # chains. Final two chunks are 256 cols so the
                        # post-matmul add+DMA tail is as short as possible.
                        chunks = [(0, 0, 512), (0, 512, 512),
                                  (1, 0, 512), (1, 512, 256), (1, 768, 256)]
                        for ci, (j, c0, cw) in enumerate(chunks):
                            psn = pspool.tile([P, 512], _FP32, tag="ps",
                                              name=f"psn_{ci}")
                            for q in range(QT):
                                nc.tensor.matmul(
                                    psn[:, :cw],
                                    w2sb[e][:, q * C + j * P:
                                            q * C + (j + 1) * P],
                                    hsb[:, q * L + c0: q * L + c0 + cw],
                                    start=(q == 0),
                                    stop=(q == QT - 1),
                                )
                            ysn = ypool.tile([P, 512], _FP16, tag="yc",
                                             name=f"ysn_{ci}")
                            nc.vector.tensor_scalar_add(
                                ysn[:, :cw], psn[:, :cw], b2col(e, j),
                            )
                            nc.sync.dma_start(
                                ys_d.ap()[b, e, :, j * L + c0:
                                          j * L + c0 + cw],
                                ysn[:, :cw],
                            )
                        continue
                    ysb = ypool.tile([P, JT * L], _FP16, tag="y",
                                     name=f"ysb_{e}_{b}")
                    for j in range(JT):
                        psy = pspool.tile([P, L], _FP32, tag="ps")
                        for q in range(QT):
                            for n in range(NT):
                                nc.tensor.matmul(
                                    psy[:, n * 512:(n + 1) * 512],
                                    w2sb[e][:, q * C + j * P: q * C + (j + 1) * P],
                                    hsb[:, q * L + n * 512: q * L + (n + 1) * 512],
                                    start=(q == 0),
                                    stop=(q == QT - 1),
                                )
                        nc.vector.tensor_scalar_add(
                            ysb[:, j * L:(j + 1) * L], psy[:], b2col(e, j),
                        )
                    nc.sync.dma_start(ys_d.ap()[b, e], ysb[:])

    nc.compile()
    return nc


_NC_CACHE = None


def _get_nc():
    global _NC_CACHE
    if _NC_CACHE is None:
        _NC_CACHE = _build()
    return _NC_CACHE


def _shard_inputs(x, W1, b1, W2, b2):
    """Full fp32 inputs -> 8 per-core input dicts, fp16, SBUF-layouted."""
    # x[b, e*C + k*128 + p, l] -> xh[b, e, p, k*L + l]
    xh = np.ascontiguousarray(
        x.astype(np.float16)
        .reshape(B, E, KT, P, L)
        .transpose(0, 1, 3, 2, 4)
        .reshape(B, E, P, KT * L)
    )
    # W1r[e, f, c]: w1h[e, k, p, f] = W1r[e, f, k*128+p]
    w1r = W1.astype(np.float16).reshape(E, F, C)
    w1h = np.ascontiguousarray(
        w1r.reshape(E, F, KT, P).transpose(0, 2, 3, 1)
    )
    # W2r[e, c, f]: w2h[e, p, q*C + c] = W2r[e, c, q*128+p]
    w2r = W2.astype(np.float16).reshape(E, C, F)
    w2h = np.ascontiguousarray(
        w2r.reshape(E, C, QT, P).transpose(0, 2, 3, 1)  # [E, q, p, c]
        .transpose(0, 2, 1, 3)                          # [E, p, q, c]
        .reshape(E, P, QT * C)
    )
    # bias pack: bh[e, p, m] = b1[e*F + m*128 + p]; bh[e, p, MT+j] = b2[...]
    b1r = b1.astype(np.float32).reshape(E, MT, P).transpose(0, 2, 1)
    b2r = b2.astype(np.float32).reshape(E, JT, P).transpose(0, 2, 1)
    bh = np.ascontiguousarray(np.concatenate([b1r, b2r], axis=2))

    in_maps = []
    for i in range(NCORES):
        es = slice(i * EPC, (i + 1) * EPC)
        in_maps.append({
            "xs": np.ascontiguousarray(xh[:, es]),
            "w1": np.ascontiguousarray(w1h[es]),
            "w2": np.ascontiguousarray(w2h[es]),
            "bs": np.ascontiguousarray(bh[es]),
        })
    return in_maps


def run(x, W1, b1, W2, b2, trace=False, **trace_kwargs):
    nc = _get_nc()
    x = np.asarray(x, dtype=np.float32)
    in_maps = _shard_inputs(x, np.asarray(W1), np.asarray(b1),
                            np.asarray(W2), np.asarray(b2))
    res = run_bass_kernel_spmd(
        nc, in_maps, core_ids=list(range(NCORES)), trace=trace, **trace_kwargs
    )
    # ys[b, e_local, p, j*L + l] per core -> y[b, e*C + j*128 + p, l]
    ys = np.concatenate([res.results[i]["ys"] for i in range(NCORES)], axis=1)
    y = (
        ys.reshape(B, E, P, JT, L)
        .transpose(0, 1, 3, 2, 4)
        .reshape(B, E * C, L)
        .astype(np.float32)
    )
    return y, res


def kernel(x, W1, b1, W2, b2):
    y, _ = run(x, W1, b1, W2, b2)
    return y.astype(np.float32)


# revision 20
# speedup vs baseline: 1.0814x; 1.0814x over previous
"""Expert-parallel grouped-MLP (MoE experts) kernel for 8 Trainium2 cores.

Problem: y = W2_e @ silu(W1_e @ x_e + b1_e) + b2_e for E=16 independent
experts (grouped 1x1 conv), B=8 batches, C=256 channels/expert, CAP=4,
L=1024 positions.

Sharding: expert-parallel - core i owns experts {2i, 2i+1}; no cross-core
communication.

v2: all operands are pre-cast to fp16 and pre-laid-out on the HOST in the
exact SBUF tile layouts, so the device does zero dtype casts and every DMA
is a fully contiguous [128, N] slab. y is written back as fp16 (host
up-casts). This halves DMA traffic, removes all DVE cast ops, roughly
halves DMA instruction count (shorter kernel-exit semaphore-clear tail),
and lets the first real matmul start ~4 us earlier.

  per (b, e) pair:  one DMA x[128, 2*1024] fp16
    layer 1: 8 m-tiles x (2k x 2n) matmuls -> PSUM[128,1024]
             ACT silu(. + b1) PSUM -> h SBUF [128, 8x1024] (fp16)
    layer 2: 2 j-tiles x (8q x 2n) accumulating matmuls -> PSUM[128,1024]
             DVE + b2 PSUM -> y SBUF fp16, one DMA out per pair
"""
import numpy as np

import concourse.tile as tile
from concourse import bacc, mybir
from concourse.bass_utils import run_bass_kernel_spmd

# Problem constants (hardcoded per contract)
B, E, C, CAP, L = 8, 16, 256, 4, 1024
F = C * CAP            # 1024 hidden per expert
NCORES = 8
EPC = E // NCORES      # 2 experts per core
P = 128                # partitions
KT = C // P            # 2 k-tiles (layer-1 contraction)
MT = F // P            # 8 m-tiles (layer-1 output partitions)
JT = C // P            # 2 j-tiles (layer-2 output partitions)
QT = F // P            # 8 q-tiles (layer-2 contraction)
NT = L // 512          # 2 n-tiles of 512 cols
NB = MT + JT           # bias columns per expert (8 m + 2 j)
N_WARMUP = 5           # dummy PE warmup matmuls

_FP32 = mybir.dt.float32
_FP16 = mybir.dt.float16


def _build():
    nc = bacc.Bacc("TRN2", target_bir_lowering=False, debug=False)

    # All tensors host-pre-arranged so each DMA is a contiguous [128, N] slab.
    xs_d = nc.dram_tensor("xs", [B, EPC, P, KT * L], _FP16, kind="ExternalInput")
    w1_d = nc.dram_tensor("w1", [EPC, KT, P, F], _FP16, kind="ExternalInput")
    w2_d = nc.dram_tensor("w2", [EPC, P, QT * C], _FP16, kind="ExternalInput")
    b_d = nc.dram_tensor("bs", [EPC, P, NB], _FP32, kind="ExternalInput")
    ys_d = nc.dram_tensor("ys", [B, EPC, P, JT * L], _FP16, kind="ExternalOutput")

    with tile.TileContext(nc) as tc:
        with (
            tc.tile_pool(name="const", bufs=1) as cpool,
            tc.tile_pool(name="x", bufs=6) as xpool,
            tc.tile_pool(name="h", bufs=2) as hpool,
            tc.tile_pool(name="y", bufs=3) as ypool,
            tc.tile_pool(name="ps", bufs=4, space="PSUM") as pspool,
        ):
            # ---- PE warmup: zero fp16 matmuls with no DMA deps ----
            wdum = cpool.tile([P, P], _FP16, tag="wdum")
            rdum = cpool.tile([P, 512], _FP16, tag="rdum")
            nc.vector.memset(wdum[:], 0.0)
            nc.vector.memset(rdum[:], 0.0)
            actdum = cpool.tile([P, 1], _FP32, tag="actdum")
            nc.scalar.activation(actdum[:], rdum[:, :1],
                                 mybir.ActivationFunctionType.Silu, bias=0.0)
            # single shared psum tile: WAW on the same bank from the same
            # engine needs no semaphores, so warmups run back-to-back and
            # keep the PE duty cycle high (earlier DVFS ramp to full clock)
            wps = pspool.tile([P, 512], _FP32, tag="ps", name="wps")
            for i in range(N_WARMUP):
                nc.tensor.matmul(wps[:], wdum[:], rdum[:],
                                 start=True, stop=True)

            # ---- weight/bias tiles ----
            # w1sb[e][k]: [128, F]   col m*128+j = output channel of m-tile m
            # w2sb[e]:    [128, QT*C] col q*C+c
            w1sb = [[cpool.tile([P, F], _FP16, tag=f"w1_{e}_{k}",
                                name=f"w1sb_{e}_{k}")
                     for k in range(KT)] for e in range(EPC)]
            w2sb = [cpool.tile([P, QT * C], _FP16, tag=f"w2_{e}",
                               name=f"w2sb_{e}")
                    for e in range(EPC)]
            bsb = cpool.tile([P, EPC * NB], _FP32, tag="b")

            def load_w1(e, k, split=1):
                for s in range(split):
                    w = F // split
                    nc.sync.dma_start(
                        w1sb[e][k][:, s * w:(s + 1) * w],
                        w1_d.ap()[e, k, :, s * w:(s + 1) * w],
                    )

            def load_w2(e):
                nc.sync.dma_start(w2sb[e][:], w2_d.ap()[e])

            def load_b(e):
                nc.sync.dma_start(
                    bsb[:, e * NB:(e + 1) * NB], b_d.ap()[e]
                )

            def load_x(b, e):
                xt = xpool.tile([P, KT * L], _FP16, tag="x",
                                name=f"x_{b}_{e}")
                nc.sync.dma_start(xt[:], xs_d.ap()[b, e])
                return xt

            # startup-critical order: first matmul needs w1[0][0] cols 0:128
            # and x[0,0] cols 0:512 -> issue exactly those two DMAs first;
            # expert-1 weights are deferred until pair 1 (needed at pair 8)
            # startup x0 chunks ride the Activation HWDGE ring so they stream
            # in parallel with the weight loads on the sync ring
            x0 = xpool.tile([P, KT * L], _FP16, tag="x", name="x_0_0")
            nc.sync.dma_start(w1sb[0][0][:, :512], w1_d.ap()[0, 0, :, :512])
            nc.scalar.dma_start(x0[:, :512], xs_d.ap()[0, 0, :, :512])
            nc.scalar.dma_start(x0[:, 512:L], xs_d.ap()[0, 0, :, 512:L])
            nc.scalar.dma_start(x0[:, L:], xs_d.ap()[0, 0, :, L:])
            nc.sync.dma_start(w1sb[0][0][:, 512:], w1_d.ap()[0, 0, :, 512:])
            load_b(0)
            load_w1(0, 1)
            load_w2(0)

            def b1col(e, m):
                return bsb[:, e * NB + m: e * NB + m + 1]

            def b2col(e, j):
                return bsb[:, e * NB + MT + j: e * NB + MT + j + 1]

            # ---- per-(expert, batch) pipeline ----
            for e in range(EPC):
                for b in range(B):
                    xsb = x0 if (e == 0 and b == 0) else load_x(b, e)
                    if e == 0 and b == 1:
                        load_w1(1, 0)
                        load_w1(1, 1)
                        load_b(1)
                        load_w2(1)

                    # layer 1: h = silu(W1 @ x + b1), h[p, m*L + l]
                    hsb = hpool.tile([P, MT * L], _FP16, tag="h")
                    for m in range(MT):
                        psh = pspool.tile([P, L], _FP32, tag="ps")
                        for k in range(KT):
                            for n in range(NT):
                                nc.tensor.matmul(
                                    psh[:, n * 512:(n + 1) * 512],
                                    w1sb[e][k][:, m * P:(m + 1) * P],
                                    xsb[:, k * L + n * 512: k * L + (n + 1) * 512],
                                    start=(k == 0),
                                    stop=(k == KT - 1),
                                )
                        nc.scalar.activation(
                            hsb[:, m * L:(m + 1) * L],
                            psh[:],
                            mybir.ActivationFunctionType.Silu,
                            bias=b1col(e, m),
                        )

                    # layer 2: y = W2 @ h + b2
                    last_pair = (e == EPC - 1 and b == B - 1)
                    if last_pair:
                        # n-outer with a SEPARATE 1-bank psum tile per chunk:
                        # DVE/DMA of earlier chunks overlap the later matmul
                        # chains. Final two chunks are 256 cols so the
                        # post-matmul add+DMA tail is as short as possible.
                        chunks = [(0, 0, 512), (0, 512, 512),
                                  (1, 0, 512), (1, 512, 256), (1, 768, 256)]
                        for ci, (j, c0, cw) in enumerate(chunks):
                            psn = pspool.tile([P, 512], _FP32, tag="ps",
                                              name=f"psn_{ci}")
                            for q in range(QT):
                                nc.tensor.matmul(
                                    psn[:, :cw],
                                    w2sb[e][:, q * C + j * P:
                                            q * C + (j + 1) * P],
                                    hsb[:, q * L + c0: q * L + c0 + cw],
                                    start=(q == 0),
                                    stop=(q == QT - 1),
                                )
                            ysn = ypool.tile([P, 512], _FP16, tag="yc",
                                             name=f"ysn_{ci}")
                            nc.vector.tensor_scalar_add(
                                ysn[:, :cw], psn[:, :cw], b2col(e, j),
                            )
                            nc.sync.dma_start(
                                ys_d.ap()[b, e, :, j * L + c0:
                                          j * L + c0 + cw],
                                ysn[:, :cw],
                            )
                        continue
                    ysb = ypool.tile([P, JT * L], _FP16, tag="y",
                                     name=f"ysb_{e}_{b}")
                    for j in range(JT):
                        psy = pspool.tile([P, L], _FP32, tag="ps")
                        for q in range(QT):
                            for n in range(NT):
                                nc.tensor.matmul(
                                    psy[:, n * 512:(n + 1) * 512],
                                    w2sb[e][:, q * C + j * P: q * C + (j + 1) * P],
                                    hsb[:, q * L + n * 512: q * L + (n + 1) * 512],
                                    start=(q == 0),
                                    stop=(q == QT - 1),
                                )
                        nc.vector.tensor_scalar_add(
                            ysb[:, j * L:(j + 1) * L], psy[:], b2col(e, j),
                        )
                    nc.sync.dma_start(ys_d.ap()[b, e], ysb[:])

    nc.compile()
    return nc


_NC_CACHE = None


def _get_nc():
    global _NC_CACHE
    if _NC_CACHE is None:
        _NC_CACHE = _build()
    return _NC_CACHE


def _shard_inputs(x, W1, b1, W2, b2):
    """Full fp32 inputs -> 8 per-core input dicts, fp16, SBUF-layouted."""
    # x[b, e*C + k*128 + p, l] -> xh[b, e, p, k*L + l]
    xh = np.ascontiguousarray(
        x.astype(np.float16)
        .reshape(B, E, KT, P, L)
        .transpose(0, 1, 3, 2, 4)
        .reshape(B, E, P, KT * L)
    )
    # W1r[e, f, c]: w1h[e, k, p, f] = W1r[e, f, k*128+p]
    w1r = W1.astype(np.float16).reshape(E, F, C)
    w1h = np.ascontiguousarray(
        w1r.reshape(E, F, KT, P).transpose(0, 2, 3, 1)
    )
    # W2r[e, c, f]: w2h[e, p, q*C + c] = W2r[e, c, q*128+p]
    w2r = W2.astype(np.float16).reshape(E, C, F)
    w2h = np.ascontiguousarray(
        w2r.reshape(E, C, QT, P).transpose(0, 2, 3, 1)  # [E, q, p, c]
        .transpose(0, 2, 1, 3)                          # [E, p, q, c]
        .reshape(E, P, QT * C)
    )
    # bias pack: bh[e, p, m] = b1[e*F + m*128 + p]; bh[e, p, MT+j] = b2[...]
    b1r = b1.astype(np.float32).reshape(E, MT, P).transpose(0, 2, 1)
    b2r = b2.astype(np.float32).reshape(E, JT, P).transpose(0, 2, 1)
    bh = np.ascontiguousarray(np.concatenate([b1r, b2r], axis=2))

    in_maps = []
    for i in range(NCORES):
        es = slice(i * EPC, (i + 1) * EPC)
        in_maps.append({
            "xs": np.ascontiguousarray(xh[:, es]),
            "w1": np.ascontiguousarray(w1h[es]),
            "w2": np.ascontiguousarray(w2h[es]),
            "bs": np.ascontiguousarray(bh[es]),
        })
    return in_maps


def run(x, W1, b1, W2, b2, trace=False, **trace_kwargs):
    nc = _get_nc()
    x = np.asarray(x, dtype=np.float32)
    in_maps = _shard_inputs(x, np.asarray(W1), np.asarray(b1),
                            np.asarray(W2), np.asarray(b2))
    res = run_bass_kernel_spmd(
        nc, in_maps, core_ids=list(range(NCORES)), trace=trace, **trace_kwargs
    )
    # ys[b, e_local, p, j*L + l] per core -> y[b, e*C + j*128 + p, l]
    ys = np.concatenate([res.results[i]["ys"] for i in range(NCORES)], axis=1)
    y = (
        ys.reshape(B, E, P, JT, L)
        .transpose(0, 1, 3, 2, 4)
        .reshape(B, E * C, L)
        .astype(np.float32)
    )
    return y, res


def kernel(x, W1, b1, W2, b2):
    y, _ = run(x, W1, b1, W2, b2)
    return y.astype(np.float32)
